# revision 1
# baseline (speedup 1.0000x reference)
"""AlphaFold-style gated MSA attention on 8 Trainium2 NeuronCores.

Batch-sharded (128 batches -> 16 per core). Full inputs in, full output out.

Math per batch b (reference):
  q = (q_data @ Wq) * hk^-0.5          [Q, H, 32]
  k = m_data @ Wk ; v = m_data @ Wv    [K, H, 32]
  S[h] = q_h k_h^T + bias[b] + nb[h]   [H, Q, K]
  w = softmax(S, axis=-1)
  wa = w @ v                            [Q, H, 32]
  gate = sigmoid(q_data @ Wg + gb)
  out = (wa * gate).reshape(Q, 256) @ Wo + o_bias

Device-side formulation (per core, layouts chosen so no transposes are
needed on-device):
  S^T[k, q] computed head-by-head from k^T/q^T projections (feature dim on
  partitions).  softmax is done unnormalized with the bias adds replaced by
  multiplies of host-precomputed exp(bias)^T ("eb") and exp(nb)^T ("en"):
      w^T = exp(S^T) * en_h * eb          (bf16)
  The V-matmul uses lhsT = [v_h | 2.0] so PSUM row 32 accumulates 2*sum_k w,
  giving the softmax denominators for free.  Normalization and gating fuse:
      ga^T = wa^T * (1 + tanh(x/2 + gb/2)) * recip(2*sum) = wa^T*sigmoid/sum
  with the per-head recip broadcast across 32 partitions by a tiny indicator
  matmul.  Output projection back to [q, 256] with o_bias added during PSUM
  evacuation.
"""

import os
import sys

sys.path.insert(0, "/opt/trn_rl_repo")

import numpy as np
import ml_dtypes
from contextlib import ExitStack

import concourse.bass as bass  # noqa: F401  (engine types)
import concourse.bacc as bacc
import concourse.mybir as mybir
import concourse.tile as tile

BF16 = ml_dtypes.bfloat16

NUM_CORES = 8
B, Q, K, A = 128, 384, 384, 256
H, HD = 8, 32  # heads, head dim
OUT = 256
BPC = B // NUM_CORES  # batches per core


PAIR_MUL = __import__("os").environ.get("PAIR_MUL", "0") == "1"
_pm = __import__("os").environ.get("PREMUL_HEADS", "0,2,4,6")
PREMUL_HEADS = tuple(int(x) for x in _pm.split(",") if x != "")
GPS_HEADS = tuple(int(x) for x in __import__("os").environ.get("GPS_HEADS", "9").split(","))


def _build_body(ctx, tc, io, bpc):
    nc = tc.nc
    f32, bf = mybir.dt.float32, mybir.dt.bfloat16
    Exp = mybir.ActivationFunctionType.Exp
    Tanh = mybir.ActivationFunctionType.Tanh
    MUL, ADD = mybir.AluOpType.mult, mybir.AluOpType.add

    const = ctx.enter_context(tc.tile_pool(name="const", bufs=1))
    lp = ctx.enter_context(tc.tile_pool(name="loads", bufs=int(__import__("os").environ.get("LP_BUFS", "5"))))
    pp = ctx.enter_context(tc.tile_pool(name="proj", bufs=int(__import__("os").environ.get("PP_BUFS", "3"))))
    wp = ctx.enter_context(tc.tile_pool(name="work", bufs=int(__import__("os").environ.get("WP_BUFS", "4"))))
    wap = ctx.enter_context(tc.tile_pool(name="wa", bufs=8))
    gp = ctx.enter_context(tc.tile_pool(name="gating", bufs=int(__import__("os").environ.get("GP_BUFS", "3"))))
    outp = ctx.enter_context(tc.tile_pool(name="outp", bufs=3))
    # PSUM: 2 x 3 banks (S^T) + 2 x 1 bank (everything else) = 8 banks.
    Sp = ctx.enter_context(tc.tile_pool(name="psum_S", bufs=2, space="PSUM"))
    sp = ctx.enter_context(tc.tile_pool(name="psum_sm", bufs=2, space="PSUM"))

    # ---- resident constants ----
    en_sb = const.tile([128, H, 3, Q], bf, tag="en")
    nc.sync.dma_start(en_sb[:], io["enT"])
    w_sb = {}
    for name in ("wq", "wk", "wv", "wg", "wo"):
        w_sb[name] = const.tile([128, 2, 256], bf, tag=name, name=name)
        nc.sync.dma_start(w_sb[name][:], io[name])
    # o_bias as a [1, 256] row plus a [1, 128] ones row for the rank-1
    # PSUM-accumulate trick (bf16 to match the other matmul operands)
    obias_row = const.tile([1, OUT], bf, tag="obias_row")
    nc.sync.dma_start(obias_row[:], io["obias_bf"])
    ones_row = const.tile([1, 128], bf, tag="ones_row")
    nc.sync.dma_start(ones_row[:], io["ind"][127:128, 0:128])
    gbh_sb = const.tile([128, 2], f32, tag="gbh")
    nc.sync.dma_start(gbh_sb[:], io["gbh"])
    if os.environ.get("ACT_WARM", "0") == "1":
        # dummy activation right after the first tiny const DMA: pulls the
        # ~2.7us ACT table load off the critical path (exp_and_others holds
        # both Exp and Tanh, so no further loads fire later)
        warm = const.tile([128, 2], f32, tag="warm")
        nc.scalar.activation(warm[:], gbh_sb[:], Exp)
    ind_sb = const.tile([128, 256], bf, tag="ind")
    nc.sync.dma_start(ind_sb[:], io["ind"])
    if os.environ.get("RBC_F32", "0") == "1":
        ind_f_sb = const.tile([8, 256], f32, tag="ind_f")
        nc.sync.dma_start(ind_f_sb[:], io["ind_f"])
    else:
        ind_f_sb = None

    def emit_tail(b, sums_bf, waA, gt):
        import contextlib
        _hp = (
            tc.high_priority()
            if os.environ.get("TAIL_HIPRI", "0") == "1"
            else contextlib.nullcontext()
        )
        with _hp:
            return _emit_tail_inner(b, sums_bf, waA, gt)

    def _emit_tail_inner(b, sums_bf, waA, gt):
        # ---- normalization + gating + output projection (batch tail) ----
        if os.environ.get("WA_F32", "0") == "1":
            sums_f = sums_bf  # already fp32
        else:
            sums_f = gp.tile([8, Q], f32, tag="sums_f", name=f"sums_f_{b}")
            _sf = os.environ.get("SF_ENG", "dve")
            if _sf == "gps":
                nc.gpsimd.tensor_copy(sums_f[:], sums_bf[:])
            elif _sf == "act":
                nc.scalar.copy(sums_f[:], sums_bf[:])
            else:
                nc.vector.tensor_copy(sums_f[:], sums_bf[:])
        rec = gp.tile([8, Q], f32, tag="rec", name=f"rec_{b}")
        nc.vector.reciprocal_approx_fast(rec[:], sums_f[:])
        if os.environ.get("RBC_F32", "0") == "1":
            recb = rec  # fp32 path: indicator matmul runs in fp32, no cast
            ind_mm = ind_f_sb
        else:
            recb = gp.tile([8, Q], bf, tag="recb", name=f"recb_{b}")
            (nc.gpsimd.tensor_copy if os.environ.get("RECB_GPS", "0") == "1" else nc.vector.tensor_copy)(recb[:], rec[:])
            ind_mm = ind_sb
        ga_tiles = []
        for j in range(2):
            _pool_r = Sp if os.environ.get("PSR_S", "0") == "1" else sp
            _tag_r = "S" if os.environ.get("PSR_S", "0") == "1" else "sm"
            psR = _pool_r.tile(
                [128, 512], f32, tag=_tag_r, name=f"psR{j}_{b}"
            )
            nc.tensor.matmul(
                psR[:, :Q],
                ind_mm[0:8, 128 * j : 128 * (j + 1)],
                recb[:],
                start=True,
                stop=True,
            )
            g2 = gp.tile([128, Q], bf, tag="g2", name=f"g2{j}_{b}")
            if os.environ.get("RBC_EVAC", "0") == "1":
                # ACT evacuates rbc so the STT runs in bf16 2x mode
                rbc_sb = gp.tile([128, Q], bf, tag="rbc_sb", name=f"rbc{j}_{b}")
                nc.scalar.copy(rbc_sb[:], psR[:, :Q])
                nc.vector.scalar_tensor_tensor(
                    g2[:], gt[:, j, :], 1.0, rbc_sb[:], op0=ADD, op1=MUL
                )
            else:
                nc.vector.scalar_tensor_tensor(
                    g2[:], gt[:, j, :], 1.0, psR[:, :Q], op0=ADD, op1=MUL
                )
            ga = gp.tile([128, Q], bf, tag="ga", name=f"ga{j}_{b}")
            (nc.gpsimd if os.environ.get("GA_GPS", "0") == "1" else nc.vector).tensor_tensor(
                ga[:], waA[j][:], g2[:], op=MUL
            )
            ga_tiles.append(ga)
        ob = outp.tile([128, 3, OUT], f32, tag="ob", name=f"ob_{b}")
        for qc in range(3):
            _pool_o = Sp if os.environ.get("PSO_S", "0") == "1" else sp
            _tag_o = "S" if os.environ.get("PSO_S", "0") == "1" else "sm"
            psO = _pool_o.tile(
                [128, 512], f32, tag=_tag_o, name=f"psO{qc}_{b}"
            )
            for j in range(2):
                nc.tensor.matmul(
                    psO[:, :OUT],
                    ga_tiles[j][:, 128 * qc : 128 * (qc + 1)],
                    w_sb["wo"][:, j, :],
                    start=(j == 0),
                    stop=False,
                )
            # rank-1 accumulate of o_bias: ones_col.T @ obias_row
            nc.tensor.matmul(
                psO[:, :OUT],
                ones_row[:],
                obias_row[:],
                start=False,
                stop=True,
            )
            (nc.scalar.copy if os.environ.get("OUT_ACT", "1") == "1" else nc.vector.tensor_copy)(ob[:, qc, :], psO[:, :OUT])
        if os.environ.get("OUT_SPLIT", "0") == "1":
            for qc in range(3):
                nc.sync.dma_start(
                    io["out"][b, 128 * qc : 128 * (qc + 1), :], ob[:, qc, :]
                )
        else:
            (nc.scalar if os.environ.get("ODMA_ACT", "0") == "1" else nc.sync).dma_start(
                io["out"][b].rearrange("(c p) o -> p c o", p=128), ob[:]
            )

    def emit_loads_proj(b):
        # ---- loads ----
        qd = lp.tile([128, 2, Q], bf, tag="qd", name=f"qd_{b}")
        nc.sync.dma_start(qd[:], io["inT"][b, :, 0:2, :])
        md = lp.tile([128, 2, Q], bf, tag="md", name=f"md_{b}")
        nc.sync.dma_start(md[:], io["inT"][b, :, 2:4, :])
        eb = lp.tile([128, 3, Q], bf, tag="eb", name=f"eb_{b}")
        nc.sync.dma_start(eb[:], io["inT"][b, :, 4:7, :])

        # ---- projections ----
        qT = pp.tile([128, 2, Q], bf, tag="qT", name=f"qT_{b}")  # [hc, j, q]
        kT = pp.tile([128, 2, Q], bf, tag="kT", name=f"kT_{b}")  # [hc, j, k]
        gt = pp.tile([128, 2, Q], bf, tag="gt", name=f"gt_{b}")
        # [k, kc, h*33+c | 2.0]; padded to 320 so every head has a 64-wide
        # lhsT window (M=64 writes initialized junk to PSUM rows 32-63,
        # letting the pair evacuation be one full-width copy).
        vv = pp.tile([128, 3, 320], bf, tag="vv", name=f"vv_{b}")
        nc.gpsimd.memset(vv[:], 2.0)

        def emit_gproj(j):
            ps = sp.tile([128, 512], f32, tag="sm", name=f"psg{j}_{b}")
            for a in range(2):
                nc.tensor.matmul(
                    ps[:, :Q],
                    w_sb["wg"][:, a, 128 * j : 128 * (j + 1)],
                    qd[:, a, :],
                    start=(a == 0),
                    stop=(a == 1),
                )
            nc.scalar.activation(
                gt[:, j, :], ps[:, :Q], Tanh, bias=gbh_sb[:, j : j + 1], scale=0.5
            )

        def emit_vproj():
            for kc in range(3):
                ps = sp.tile([128, 512], f32, tag="sm", name=f"psv{kc}_{b}")
                for a in range(2):
                    nc.tensor.matmul(
                        ps[:, :256],
                        md[:, a, 128 * kc : 128 * (kc + 1)],
                        w_sb["wv"][:, a, :],
                        start=(a == 0),
                        stop=(a == 1),
                    )
                _ev = (
                    nc.scalar.copy
                    if os.environ.get("VV_ACT", "1") == "1"
                    else nc.vector.tensor_copy
                )
                _ev(
                    vv[:, kc, 0:264].rearrange("p (h c) -> p h c", c=33)[:, :, 0:32],
                    ps[:, :256].rearrange("p (h c) -> p h c", c=32),
                )

        for j in range(2):
            ps = sp.tile([128, 512], f32, tag="sm", name=f"psq{j}_{b}")
            for a in range(2):
                nc.tensor.matmul(
                    ps[:, :Q],
                    w_sb["wq"][:, a, 128 * j : 128 * (j + 1)],
                    qd[:, a, :],
                    start=(a == 0),
                    stop=(a == 1),
                )
            (nc.scalar.copy if os.environ.get("QK_ACT", "0") in ("1", "q") else nc.vector.tensor_copy)(qT[:, j, :], ps[:, :Q])
            ps = sp.tile([128, 512], f32, tag="sm", name=f"psk{j}_{b}")
            for a in range(2):
                nc.tensor.matmul(
                    ps[:, :Q],
                    w_sb["wk"][:, a, 128 * j : 128 * (j + 1)],
                    md[:, a, :],
                    start=(a == 0),
                    stop=(a == 1),
                )
            (nc.scalar.copy if os.environ.get("QK_ACT", "0") == "1" else nc.vector.tensor_copy)(kT[:, j, :], ps[:, :Q])
            if os.environ.get("PROJ_ORDER", "a") == "a":
                emit_gproj(j)
        if os.environ.get("PROJ_ORDER", "a") == "b":
            # v first (needed by the first V-matmul early in the heads);
            # gate last (not needed until the batch tail)
            emit_vproj()
            emit_gproj(0)
            emit_gproj(1)
        else:
            emit_vproj()
        # Precompute en*eb for even heads on the otherwise-idle GPSIMD —
        # depends only on the eb load, so it runs ahead of the critical path.
        pw = None
        if PREMUL_HEADS:
            pw = wp.tile(
                [128, len(PREMUL_HEADS), 3, Q], bf, tag="pw", name=f"pw_{b}"
            )
            for i, hx in enumerate(PREMUL_HEADS):
                nc.gpsimd.tensor_tensor(pw[:, i], en_sb[:, hx], eb[:], op=MUL)
        # pair p's (2*sum) rows are DMA'd (engines cannot do partition-strided
        # APs) from the evacuated wa tiles into rows {2p, 2p+1} of sums_bf.
        sums_bf = gp.tile([8, Q], bf, tag="sums_bf", name=f"sums_bf_{b}")
        waA = [
            wap.tile([128, Q], bf, tag="waA", name=f"waA{j}_{b}") for j in range(2)
        ]
        return dict(
            qd=qd, md=md, eb=eb, qT=qT, kT=kT, gt=gt, vv=vv, pw=pw,
            sums_bf=sums_bf, waA=waA, wa_tiles=[], psW=None,
        )

    def emit_heads(b, st, heads):
        qT, kT, eb, vv = st["qT"], st["kT"], st["eb"], st["vv"]
        sums_bf, waA = st["sums_bf"], st["waA"]
        for h in heads:
            j, hh, p = h // 4, h % 4, h % 2
            psS = Sp.tile([128, 1536], f32, tag="S")
            for kc in range(3):
                nc.tensor.matmul(
                    psS[:, 512 * kc : 512 * kc + Q],
                    kT[32 * hh : 32 * (hh + 1), j, 128 * kc : 128 * (kc + 1)],
                    qT[32 * hh : 32 * (hh + 1), j, :],
                    start=True,
                    stop=True,
                    tile_position=(32 * hh, 0),
                )
            sview = psS[:].rearrange("p (c x) -> p c x", x=512)[:, :, :Q]
            if PAIR_MUL:
                if p == 0:
                    st["es2"] = wp.tile(
                        [128, 2, 3, Q], bf, tag="es2", name=f"es2_{h}_{b}"
                    )
                nc.scalar.activation(st["es2"][:, p], sview, Exp)
            else:
                w = wp.tile([128, 3, Q], bf, tag="w", bufs=int(os.environ.get("W_BUFS", "4")))
                if os.environ.get("INPLACE_MUL", "0") == "1":
                    nc.scalar.activation(w[:], sview, Exp)
                    eng = nc.gpsimd if hh in GPS_HEADS else nc.vector
                    eng.tensor_tensor(w[:], w[:], en_sb[:, h], op=MUL)
                    eng.tensor_tensor(w[:], w[:], eb[:], op=MUL)
                elif h in PREMUL_HEADS:
                    es = wp.tile([128, 3, Q], bf, tag="es", bufs=int(os.environ.get("ES_BUFS", "4")))
                    nc.scalar.activation(es[:], sview, Exp)
                    nc.vector.tensor_tensor(
                        w[:], es[:], st["pw"][:, PREMUL_HEADS.index(h)], op=MUL
                    )
                else:
                    es = wp.tile([128, 3, Q], bf, tag="es", bufs=int(os.environ.get("ES_BUFS", "4")))
                    nc.scalar.activation(es[:], sview, Exp)
                    eng = nc.gpsimd if hh in GPS_HEADS else nc.vector
                    eng.tensor_tensor(w[:], es[:], en_sb[:, h], op=MUL)
                    eng.tensor_tensor(w[:], w[:], eb[:], op=MUL)
            if p == 0:
                st["psW"] = sp.tile([128, 512], f32, tag="sm", name=f"psW{h}_{b}")
            psW = st["psW"]
            if PAIR_MUL:
                if p == 1:
                    w2 = wp.tile([128, 2, 3, Q], bf, tag="w2", name=f"w2_{h}_{b}")
                    nc.vector.tensor_tensor(
                        w2[:], st["es2"][:], en_sb[:, h - 1 : h + 1], op=MUL
                    )
                    nc.vector.tensor_tensor(
                        w2[:],
                        w2[:],
                        eb[:].unsqueeze(1).broadcast_to((128, 2, 3, Q)),
                        op=MUL,
                    )
                    for hp in range(2):
                        hx = h - 1 + hp
                        for kc in range(3):
                            nc.tensor.matmul(
                                psW[64 * hp : 64 * hp + 64, :Q],
                                vv[:, kc, 33 * hx : 33 * hx + 64],
                                w2[:, hp, kc, :],
                                start=(kc == 0),
                                stop=(kc == 2),
                            )
            else:
                for kc in range(3):
                    nc.tensor.matmul(
                        psW[64 * p : 64 * p + 64, :Q],
                        vv[:, kc, 33 * h : 33 * h + 64],
                        w[:, kc, :],
                        start=(kc == 0),
                        stop=(kc == 2),
                    )
            if p == 1:
                # evacuate both heads (incl. the 2*sum rows 32 and 96)
                wa = wap.tile(
                    [128, Q],
                    f32 if os.environ.get("WA_F32", "0") == "1" else bf,
                    tag="wa",
                )
                _wa_mode = os.environ.get("WA_EV", "dve")
                if _wa_mode == "act":
                    ev = nc.scalar
                elif _wa_mode == "dve":
                    ev = nc.vector
                else:
                    ev = nc.scalar if (h // 2) % 2 else nc.vector
                if ev is nc.scalar:
                    ev_inst = ev.copy(wa[:, :], psW[:, :Q])
                else:
                    ev_inst = ev.tensor_copy(wa[:, :], psW[:, :Q])
                pr = 2 * (h // 2)
                _dq = nc.scalar if os.environ.get("SDMA_ACT", "0") == "1" else nc.sync
                if os.environ.get("MERGE_DMA", "0") == "1":
                    # merged per-pair DMAs with explicit deps on the evac
                    # (Tile's tracker misses deps for strided partition APs)
                    import bass_rust as _br
                    from concourse.tile_rust import add_dep_helper as _adh

                    d1 = _dq.dma_start(sums_bf[pr : pr + 2, :], wa[32:97:64, :])
                    _adh(d1.ins, ev_inst.ins, reason="strided sums read of wa")
                    jj = h // 4
                    r_dst = 32 * ((h - 1) % 4)
                    srcap = _br.AP(
                        wa.tensor, wa.offset, [[64 * Q, 2], [Q, 32], [1, Q]]
                    )
                    dstap = _br.AP(
                        waA[jj].tensor,
                        waA[jj].offset + r_dst * Q,
                        [[32 * Q, 2], [Q, 32], [1, Q]],
                    )
                    d2 = _dq.dma_start(dstap, srcap)
                    _adh(d2.ins, ev_inst.ins, reason="pair rearrange read of wa")
                else:
                    _dq.dma_start(sums_bf[pr : pr + 1, :], wa[32:33, :])
                    _dq.dma_start(sums_bf[pr + 1 : pr + 2, :], wa[96:97, :])
                    # rearrange both heads into the gate-aligned chunk tile
                    # (engines cannot shift partition base; DMA can)
                    for hx, r0 in ((h - 1, 0), (h, 64)):
                        jj, hh2 = hx // 4, hx % 4
                        _dq.dma_start(
                            waA[jj][32 * hh2 : 32 * hh2 + 32, :], wa[r0 : r0 + 32, :]
                        )
                st["wa_tiles"].append(wa)

    if os.environ.get("LEAD2", "0") == "1":
        # two-batch software-pipeline lead: proj(b+2) is emitted right after
        # heads(b), so every projection has a full head-phase of slack
        states = {0: emit_loads_proj(0), 1: emit_loads_proj(1)}
        prev = None
        for b in range(bpc):
            if prev is not None:
                emit_tail(b - 1, prev["sums_bf"], prev["waA"], prev["gt"])
            st = states.pop(b)
            emit_heads(b, st, range(0, 8))
            if b + 2 < bpc:
                states[b + 2] = emit_loads_proj(b + 2)
            prev = st
        emit_tail(bpc - 1, prev["sums_bf"], prev["waA"], prev["gt"])
    else:
        # Software pipeline: loads+projections of batch b, then the
        # latency-heavy tail of batch b-1 (overlapping this batch's heads).
        prev = None
        for b in range(bpc):
            st = emit_loads_proj(b)
            if prev is not None:
                emit_tail(b - 1, prev["sums_bf"], prev["waA"], prev["gt"])
            emit_heads(b, st, range(0, 8))
            prev = st
        emit_tail(bpc - 1, prev["sums_bf"], prev["waA"], prev["gt"])


def build(bpc=BPC):
    nc = bacc.Bacc(
        "TRN2",
        target_bir_lowering=False,
        debug=False,
        enable_asserts=False,
        num_devices=NUM_CORES,
    )
    f32, bf = mybir.dt.float32, mybir.dt.bfloat16
    io = {
        "inT": nc.dram_tensor("inT", [bpc, 128, 7, Q], bf, kind="ExternalInput").ap(),
        "enT": nc.dram_tensor("enT", [128, H, 3, Q], bf, kind="ExternalInput").ap(),
        "wq": nc.dram_tensor("wq", [128, 2, 256], bf, kind="ExternalInput").ap(),
        "wk": nc.dram_tensor("wk", [128, 2, 256], bf, kind="ExternalInput").ap(),
        "wv": nc.dram_tensor("wv", [128, 2, 256], bf, kind="ExternalInput").ap(),
        "wg": nc.dram_tensor("wg", [128, 2, 256], bf, kind="ExternalInput").ap(),
        "wo": nc.dram_tensor("wo", [128, 2, 256], bf, kind="ExternalInput").ap(),
        "obias_bf": nc.dram_tensor("obias_bf", [1, OUT], bf, kind="ExternalInput").ap(),
        "gbh": nc.dram_tensor("gbh", [128, 2], f32, kind="ExternalInput").ap(),
        "ind": nc.dram_tensor("ind", [128, 256], bf, kind="ExternalInput").ap(),
        "ind_f": nc.dram_tensor("ind_f", [8, 256], f32, kind="ExternalInput").ap(),
        "out": nc.dram_tensor("out", [bpc, Q, OUT], f32, kind="ExternalOutput").ap(),
    }
    with tile.TileContext(nc) as tc:
        with ExitStack() as ctx:
            _build_body(ctx, tc, io, bpc)
    nc.compile()
    return nc


def _prep_inputs(
    q_data,
    m_data,
    bias,
    nonbatched_bias,
    q_weights,
    k_weights,
    v_weights,
    o_weights,
    o_bias,
    gating_w,
    gating_b,
):
    """Host-side preprocessing into the DMA-friendly device layouts."""
    scale = q_weights.shape[-1] ** -0.5

    def featT(x):  # [B, S, A] -> [B, 128, A//128, S]
        b, s, a = x.shape
        t = x.transpose(0, 2, 1).reshape(b, a // 128, 128, s).transpose(0, 2, 1, 3)
        return np.ascontiguousarray(t.astype(BF16))

    qdT = featT(q_data)
    mdT = featT(m_data)
    eb = np.exp(bias[:, 0].transpose(0, 2, 1).astype(np.float32))  # [B, K, Q]
    ebT = np.ascontiguousarray(
        eb.reshape(B, 3, 128, Q).transpose(0, 2, 1, 3).astype(BF16)
    )
    en = np.exp(nonbatched_bias.transpose(0, 2, 1).astype(np.float32))  # [H, K, Q]
    enT = np.ascontiguousarray(
        en.reshape(H, 3, 128, Q).transpose(2, 0, 1, 3).astype(BF16)
    )

    def wmat(w, s=1.0):  # [A, H, hd] -> [128, 2, 256]
        m = (w.reshape(A, H * HD) * s).astype(BF16)
        return np.ascontiguousarray(m.reshape(2, 128, 256).transpose(1, 0, 2))

    wq = wmat(q_weights, scale)
    wk = wmat(k_weights)
    wv = wmat(v_weights)
    wg = wmat(gating_w)
    wo = np.ascontiguousarray(
        o_weights.reshape(256, 256).astype(BF16).reshape(2, 128, 256).transpose(1, 0, 2)
    )
    obias_bf = np.ascontiguousarray(o_bias.astype(BF16).reshape(1, OUT))
    gbh = np.ascontiguousarray(
        (0.5 * gating_b.reshape(H * HD).astype(np.float32)).reshape(2, 128).T
    )
    # indicator for the recip broadcast: row h selects the 32 output
    # partitions belonging to head h.
    ind = np.zeros((128, 256), dtype=BF16)
    for h in range(8):
        ind[h, 32 * h : 32 * (h + 1)] = 1.0
    ind[127, :] = 1.0  # ones row for the o_bias rank-1 matmul
    ind_f = np.ascontiguousarray(ind[0:8].astype(np.float32))
    inT = np.ascontiguousarray(np.concatenate([qdT, mdT, ebT], axis=2))
    return dict(
        inT=inT, enT=enT, wq=wq, wk=wk, wv=wv, wg=wg, wo=wo,
        obias_bf=obias_bf, gbh=gbh, ind=ind, ind_f=ind_f,
    )


_NC_CACHE = {}


def kernel(**inputs):
    from concourse.bass_utils import run_bass_kernel_spmd

    full = _prep_inputs(**{k: np.asarray(v) for k, v in inputs.items()})
    if BPC not in _NC_CACHE:
        _NC_CACHE[BPC] = build(BPC)
    nc = _NC_CACHE[BPC]

    shared = {k: full[k] for k in ("enT", "wq", "wk", "wv", "wg", "wo", "obias_bf", "gbh", "ind", "ind_f")}
    in_maps = []
    for c in range(NUM_CORES):
        sl = slice(c * BPC, (c + 1) * BPC)
        in_maps.append(dict(inT=full["inT"][sl], **shared))

    trace = bool(int(os.environ.get("BASS_KERNEL_TRACE", "0")))
    if trace:
        try:
            from antenv.axon_hooks import get_axon_ntff_profile_hook  # noqa: F401
        except Exception:
            trace = False
    import time

    t0 = time.time()
    res = run_bass_kernel_spmd(
        nc, in_maps, core_ids=list(range(NUM_CORES)), trace=trace
    )
    kernel.last_run_wall_s = time.time() - t0
    if trace and res.exec_time_ns is not None:
        print(f"HW exec time: {res.exec_time_ns} ns")
        kernel.last_exec_time_ns = res.exec_time_ns
    out = np.concatenate([r["out"] for r in res.results], axis=0)
    return out.astype(np.float32)



# revision 11
# speedup vs baseline: 1.0146x; 1.0146x over previous
"""AlphaFold-style gated MSA attention on 8 Trainium2 NeuronCores.

Batch-sharded (128 batches -> 16 per core). Full inputs in, full output out.

Math per batch b (reference):
  q = (q_data @ Wq) * hk^-0.5          [Q, H, 32]
  k = m_data @ Wk ; v = m_data @ Wv    [K, H, 32]
  S[h] = q_h k_h^T + bias[b] + nb[h]   [H, Q, K]
  w = softmax(S, axis=-1)
  wa = w @ v                            [Q, H, 32]
  gate = sigmoid(q_data @ Wg + gb)
  out = (wa * gate).reshape(Q, 256) @ Wo + o_bias

Device-side formulation (per core):
  - All projections run as fp8e4 DoubleRow matmuls (contraction 256 = 2
    k-tiles of 128 in one instruction at 0.5 cycles/row).  Host pre-scales
    qd/md by 1/8 and the weights by 64 to center fp8 magnitudes; the PSUM
    evacuations scale by 1/8 (DVE tensor_scalar) to restore true values.
  - The additive biases are folded into the S PSUM accumulation: host ships
    fused[b,h] = (bias[b] + nb[h])^T in fp8, and per (head, k-chunk) a
    DoubleRow identity matmul (lhsT = [I|0]) adds it to S^T.  exp() then
    reads the complete logits from PSUM and writes the softmax numerator
    directly as fp8 (ACT bias applies a -2 shift to keep exp() inside the
    fp8e4 range; the shift cancels in the softmax normalization).
  - The V matmul uses lhsT = [v_h | 2.0] packed per head so PSUM row 32
    accumulates 2*sum_k w (softmax denominators for free), with kc0/kc1 as
    one fp8 DoubleRow matmul and kc2 plain fp8.
  - The per-pair psW tiles are evacuated into one waBIG tile; a single
    descriptor-merged DMA gathers the 8 denominator rows and two DMAs
    rearrange the head blocks into gate-aligned waA tiles (engines cannot
    shift partitions; DMA can, and merging keeps HWDGE occupancy low).
  - Normalization, gating and the output projection follow the rank-1
    tricks of the bf16 version: an indicator matmul broadcasts 1/(2*sum)
    across each head's 32 partitions, gate fuses via scalar_tensor_tensor,
    and o_bias rides a ones-row rank-1 matmul into the PSUM group.
"""

import os
import sys

sys.path.insert(0, "/opt/trn_rl_repo")

import numpy as np
import ml_dtypes
from contextlib import ExitStack

import concourse.bass as bass  # noqa: F401  (engine types)
import concourse.bacc as bacc
import concourse.mybir as mybir
import concourse.tile as tile

BF16 = ml_dtypes.bfloat16
F8 = ml_dtypes.float8_e4m3fn

NUM_CORES = 8
B, Q, K, A = 128, 384, 384, 256
H, HD = 8, 32  # heads, head dim
OUT = 256
BPC = B // NUM_CORES  # batches per core

SHIFT = 3.0  # exp(logit - SHIFT): keeps fp8e4 w below saturation

DR = mybir.MatmulPerfMode.DoubleRow


def _env(name, default):
    return os.environ.get(name, default)


def _build_body(ctx, tc, io, bpc):
    nc = tc.nc
    f32, bf, f8 = mybir.dt.float32, mybir.dt.bfloat16, mybir.dt.float8e4
    Exp = mybir.ActivationFunctionType.Exp
    Tanh = mybir.ActivationFunctionType.Tanh
    MUL, ADD = mybir.AluOpType.mult, mybir.AluOpType.add

    import bass_rust as _br
    from concourse.tile_rust import add_dep_helper as _adh

    const = ctx.enter_context(tc.tile_pool(name="const", bufs=1))
    lp = ctx.enter_context(tc.tile_pool(name="loads", bufs=int(_env("LP_BUFS", "3"))))
    pp = ctx.enter_context(tc.tile_pool(name="proj", bufs=int(_env("PP_BUFS", "3"))))
    wp = ctx.enter_context(tc.tile_pool(name="work", bufs=int(_env("WP_BUFS", "4"))))
    wap = ctx.enter_context(tc.tile_pool(name="wa", bufs=int(_env("WA_BUFS", "2"))))
    gp = ctx.enter_context(tc.tile_pool(name="gating", bufs=int(_env("GP_BUFS", "3"))))
    outp = ctx.enter_context(tc.tile_pool(name="outp", bufs=3))
    Sp = ctx.enter_context(tc.tile_pool(name="psum_S", bufs=2, space="PSUM"))
    sp = ctx.enter_context(tc.tile_pool(name="psum_sm", bufs=2, space="PSUM"))

    VV_BUFS = int(_env("VV_BUFS", "3"))

    # ---- resident constants ----
    w_sb = {}
    for name in ("wq", "wg"):
        w_sb[name] = const.tile([128, 2, 256], f8, tag=name, name=name)
        nc.sync.dma_start(w_sb[name][:], io[name])
    for name in ("wk", "wv"):
        w_sb[name] = const.tile([128, 2, 256], bf, tag=name, name=name)
        nc.sync.dma_start(w_sb[name][:], io[name])
    w_sb["wo"] = const.tile([128, 2, 256], bf, tag="wo", name="wo")
    nc.sync.dma_start(w_sb["wo"][:], io["wo"])
    obias_row = const.tile([1, OUT], bf, tag="obias_row")
    nc.sync.dma_start(obias_row[:], io["obias_bf"])
    ones_row = const.tile([1, 128], bf, tag="ones_row")
    nc.sync.dma_start(ones_row[:], io["ind"][127:128, 0:128])
    gbh_sb = const.tile([128, 2], f32, tag="gbh")
    nc.sync.dma_start(gbh_sb[:], io["gbh"])
    shift_sb = const.tile([128, 1], f32, tag="shift")
    nc.gpsimd.memset(shift_sb[:], -SHIFT)
    # dummy activation early: pulls the ACT table load off the critical path
    warm = const.tile([128, 2], f32, tag="warm")
    nc.scalar.activation(warm[:], gbh_sb[:], Exp)
    ind_sb = const.tile([128, 256], bf, tag="ind")
    nc.sync.dma_start(ind_sb[:], io["ind"])
    # [I | 0] for the DoubleRow bias fold
    id2 = const.tile([128, 2, 128], f8, tag="id2")
    id2_dma = nc.sync.dma_start(id2[:], io["id2"])

    # ---- stable-buffer init: vv tiles carry 2.0 in the sum column slots ----
    vv_init = []
    for i in range(VV_BUFS):
        t = pp.tile([128, 3, 320], bf, tag="vv", name=f"vv_init{i}", bufs=VV_BUFS)
        nc.gpsimd.memset(t[:], 2.0)
        vv_init.append(t)

    def emit_loads(b):
        ld = lp.tile([128, 50, Q], f8, tag="ld", name=f"ld_{b}")
        nc.sync.dma_start(ld[:], io["inT"][b])
        ldb = lp.tile([128, 2, Q], bf, tag="ldb", name=f"ldb_{b}")
        nc.sync.dma_start(ldb[:], io["inTb"][b])
        return ld, ldb

    def emit_proj(b, ld, ldb):
        # q/gate: fp8 DoubleRow (evac rescales 1/8); k/v: bf16 (accuracy).
        qT = pp.tile([128, 2, Q], bf, tag="qT", name=f"qT_{b}")
        kT = pp.tile([128, 2, Q], bf, tag="kT", name=f"kT_{b}")
        gt = pp.tile([128, 2, Q], bf, tag="gt", name=f"gt_{b}")
        vv = pp.tile([128, 3, 320], bf, tag="vv", name=f"vv_{b}", bufs=VV_BUFS)
        qd = ld[:, 0:2, :]
        for j in range(2):
            ps = sp.tile([128, 512], f32, tag="sm", name=f"psq{j}_{b}")
            nc.tensor.matmul(
                ps[:, :Q], w_sb["wq"][:, :, 128 * j : 128 * (j + 1)], qd,
                start=True, stop=True, perf_mode=DR,
            )
            nc.vector.tensor_scalar_mul(qT[:, j, :], ps[:, :Q], 0.125)
            ps = sp.tile([128, 512], f32, tag="sm", name=f"psk{j}_{b}")
            for a in range(2):
                nc.tensor.matmul(
                    ps[:, :Q],
                    w_sb["wk"][:, a, 128 * j : 128 * (j + 1)],
                    ldb[:, a, :],
                    start=(a == 0), stop=(a == 1),
                )
            nc.vector.tensor_copy(kT[:, j, :], ps[:, :Q])
            ps = sp.tile([128, 512], f32, tag="sm", name=f"psg{j}_{b}")
            nc.tensor.matmul(
                ps[:, :Q], w_sb["wg"][:, :, 128 * j : 128 * (j + 1)], qd,
                start=True, stop=True, perf_mode=DR,
            )
            nc.scalar.activation(
                gt[:, j, :], ps[:, :Q], Tanh, bias=gbh_sb[:, j : j + 1], scale=0.0625
            )
        for kc in range(3):
            ps = sp.tile([128, 512], f32, tag="sm", name=f"psv{kc}_{b}")
            for a in range(2):
                nc.tensor.matmul(
                    ps[:, :256],
                    ldb[:, a, 128 * kc : 128 * (kc + 1)],
                    w_sb["wv"][:, a, :],
                    start=(a == 0), stop=(a == 1),
                )
            nc.vector.tensor_copy(
                vv[:, kc, 0:264].rearrange("p (h c) -> p h c", c=33)[:, :, 0:32],
                ps[:, :256].rearrange("p (h c) -> p h c", c=32),
            )
        sums_bf = gp.tile([8, Q], bf, tag="sums_bf", name=f"sums_bf_{b}")
        waBIG = wap.tile([128, 4, Q], bf, tag="waBIG", name=f"waBIG_{b}")
        waA = [
            gp.tile([128, Q], bf, tag=f"waA{j}", name=f"waA{j}_{b}") for j in range(2)
        ]
        return dict(
            ld=ld, qT=qT, kT=kT, gt=gt, vv=vv,
            sums_bf=sums_bf, waBIG=waBIG, waA=waA,
            wa_evac=[None] * 4, psW=None,
        )

    def emit_heads(b, st, first=False):
        ld, qT, kT, vv, waBIG = st["ld"], st["qT"], st["kT"], st["vv"], st["waBIG"]
        for h in range(8):
            j, hh, p, pw = h // 4, h % 4, h // 2, h % 2
            psS = Sp.tile([128, 1536], f32, tag="S", name=f"psS{h}_{b}")
            for kc in range(3):
                # DoubleRow bias fold: psS = fused[b,h,kc] (+ 0*junk)
                s0 = 2 + 3 * h + kc
                fm = nc.tensor.matmul(
                    psS[:, 512 * kc : 512 * kc + Q],
                    id2[:],
                    ld[:, s0 : s0 + 25 : 24, :],
                    start=True, stop=False, perf_mode=DR,
                )
                if first and h == 0:
                    _adh(fm.ins, id2_dma.ins, reason="id2 load before first fold")
                # S^T accumulate on top
                nc.tensor.matmul(
                    psS[:, 512 * kc : 512 * kc + Q],
                    kT[32 * hh : 32 * (hh + 1), j, 128 * kc : 128 * (kc + 1)],
                    qT[32 * hh : 32 * (hh + 1), j, :],
                    start=False, stop=True,
                    tile_position=(32 * hh, 0),
                )
            sview = psS[:].rearrange("p (c x) -> p c x", x=512)[:, :, :Q]
            w4 = wp.tile([128, 3, Q], bf, tag="w4", name=f"w4_{h}_{b}")
            nc.scalar.activation(w4[:], sview, Exp, bias=shift_sb[:])
            if pw == 0:
                st["psW"] = sp.tile([128, 512], f32, tag="sm", name=f"psW{h}_{b}")
            psW = st["psW"]
            for kc in range(3):
                nc.tensor.matmul(
                    psW[64 * pw : 64 * pw + 64, :Q],
                    vv[:, kc, 33 * h : 33 * h + 64],
                    w4[:, kc, :],
                    start=(kc == 0), stop=(kc == 2),
                )
            if pw == 1:
                ev = nc.vector.tensor_copy(waBIG[:, p, :], psW[:, :Q])
                st["wa_evac"][p] = ev

    # raw-AP gather bookkeeping for WAR insurance on waBIG reuse
    last_gathers = {}

    def emit_tail(b, st):
        waBIG, sums_bf, waA, gt = st["waBIG"], st["sums_bf"], st["waA"], st["gt"]
        evacs = st["wa_evac"]
        parity = b % int(_env("WA_BUFS", "2"))
        # one DMA: the 8 denominator rows (2*sum) -> sums_bf [8, Q].
        # Row order is (r, p): row i holds head 2*(i%4) + i//4; the host ind
        # matrix is permuted to match.
        src = _br.AP(
            waBIG.tensor,
            waBIG.offset + 32 * 4 * Q,
            [[64 * 4 * Q, 2], [Q, 4], [1, Q]],
        )
        dst = _br.AP(sums_bf.tensor, sums_bf.offset, [[Q, 8], [1, Q]])
        d = nc.sync.dma_start(dst, src)
        for ev in evacs:
            _adh(d.ins, ev.ins, reason="sums gather reads waBIG")
        gathers = [d]
        # cast + reciprocal + bf16 rebroadcast source
        sums_f = gp.tile([8, Q], f32, tag="sums_f", name=f"sums_f_{b}")
        sc = nc.vector.tensor_copy(sums_f[:], sums_bf[:])
        _adh(sc.ins, d.ins, reason="sums_bf filled by gather")
        rec = gp.tile([8, Q], f32, tag="rec", name=f"rec_{b}")
        nc.vector.reciprocal_approx_fast(rec[:], sums_f[:])
        recb = gp.tile([8, Q], bf, tag="recb", name=f"recb_{b}")
        nc.vector.tensor_copy(recb[:], rec[:])
        # head blocks into gate-aligned waA tiles (tile-slice DMAs; partition
        # shifts are DMA-only).  head h: waBIG[64*(h%2):+32, h//2] ->
        # waA[h//4][32*(h%4):+32]
        wadmas = []
        for h in range(8):
            j, hh, p, r = h // 4, h % 4, h // 2, h % 2
            dj = nc.sync.dma_start(
                waA[j][32 * hh : 32 * hh + 32, :],
                waBIG[64 * r : 64 * r + 32, p, :],
            )
            wadmas.append(dj)
            gathers.append(dj)
        last_gathers[parity] = gathers

        ga_tiles = []
        for j in range(2):
            psR = sp.tile([128, 512], f32, tag="sm", name=f"psR{j}_{b}")
            nc.tensor.matmul(
                psR[:, :Q], ind_sb[0:8, 128 * j : 128 * (j + 1)], recb[:],
                start=True, stop=True,
            )
            g2 = gp.tile([128, Q], bf, tag="g2", name=f"g2{j}_{b}")
            nc.vector.scalar_tensor_tensor(
                g2[:], gt[:, j, :], 1.0, psR[:, :Q], op0=ADD, op1=MUL
            )
            ga = gp.tile([128, Q], bf, tag="ga", name=f"ga{j}_{b}")
            gm = nc.vector.tensor_tensor(ga[:], waA[j][:], g2[:], op=MUL)
            ga_tiles.append(ga)
        ob = outp.tile([128, 3, OUT], f32, tag="ob", name=f"ob_{b}")
        for qc in range(3):
            psO = sp.tile([128, 512], f32, tag="sm", name=f"psO{qc}_{b}")
            for j in range(2):
                nc.tensor.matmul(
                    psO[:, :OUT],
                    ga_tiles[j][:, 128 * qc : 128 * (qc + 1)],
                    w_sb["wo"][:, j, :],
                    start=(j == 0), stop=False,
                )
            nc.tensor.matmul(
                psO[:, :OUT], ones_row[:], obias_row[:], start=False, stop=True
            )
            nc.vector.tensor_copy(ob[:, qc, :], psO[:, :OUT])
        nc.sync.dma_start(io["out"][b].rearrange("(c p) o -> p c o", p=128), ob[:])

    def guard_evacs(st, b):
        # WAR insurance: this batch's waBIG writes wait for the gathers that
        # read the buffer two batches ago (raw-AP reads are invisible to the
        # tile tracker).
        parity = b % int(_env("WA_BUFS", "2"))
        old = last_gathers.get(parity)
        if old:
            for ev in st["wa_evac"]:
                for g in old:
                    _adh(ev.ins, g.ins, reason="waBIG reuse after raw gather")

    # Software pipeline: loads+projections of batch b, then the latency-heavy
    # tail of batch b-1 (overlapping this batch's heads).
    prev = None
    for b in range(bpc):
        ld, ldb = emit_loads(b)
        st = emit_proj(b, ld, ldb)
        if prev is not None:
            emit_tail(b - 1, prev)
        emit_heads(b, st, first=(b == 0))
        guard_evacs(st, b)
        prev = st
    emit_tail(bpc - 1, prev)


def build(bpc=BPC):
    nc = bacc.Bacc(
        "TRN2",
        target_bir_lowering=False,
        debug=False,
        enable_asserts=False,
        num_devices=NUM_CORES,
    )
    f32, bf, f8 = mybir.dt.float32, mybir.dt.bfloat16, mybir.dt.float8e4
    io = {
        "inT": nc.dram_tensor("inT", [bpc, 128, 50, Q], f8, kind="ExternalInput").ap(),
        "inTb": nc.dram_tensor("inTb", [bpc, 128, 2, Q], bf, kind="ExternalInput").ap(),
        "wq": nc.dram_tensor("wq", [128, 2, 256], f8, kind="ExternalInput").ap(),
        "wk": nc.dram_tensor("wk", [128, 2, 256], bf, kind="ExternalInput").ap(),
        "wv": nc.dram_tensor("wv", [128, 2, 256], bf, kind="ExternalInput").ap(),
        "wg": nc.dram_tensor("wg", [128, 2, 256], f8, kind="ExternalInput").ap(),
        "wo": nc.dram_tensor("wo", [128, 2, 256], bf, kind="ExternalInput").ap(),
        "obias_bf": nc.dram_tensor("obias_bf", [1, OUT], bf, kind="ExternalInput").ap(),
        "gbh": nc.dram_tensor("gbh", [128, 2], f32, kind="ExternalInput").ap(),
        "ind": nc.dram_tensor("ind", [128, 256], bf, kind="ExternalInput").ap(),
        "id2": nc.dram_tensor("id2", [128, 2, 128], f8, kind="ExternalInput").ap(),
        "out": nc.dram_tensor("out", [bpc, Q, OUT], f32, kind="ExternalOutput").ap(),
    }
    with tile.TileContext(nc) as tc:
        with ExitStack() as ctx:
            _build_body(ctx, tc, io, bpc)
    nc.compile()
    return nc


def _prep_inputs(
    q_data,
    m_data,
    bias,
    nonbatched_bias,
    q_weights,
    k_weights,
    v_weights,
    o_weights,
    o_bias,
    gating_w,
    gating_b,
):
    """Host-side preprocessing into the DMA-friendly device layouts."""
    scale = q_weights.shape[-1] ** -0.5

    def featT(x, s):  # [B, S, A] -> [B, 128, A//128, S] scaled
        b, sl, a = x.shape
        t = x.transpose(0, 2, 1).reshape(b, a // 128, 128, sl).transpose(0, 2, 1, 3)
        return np.ascontiguousarray((t * s).astype(F8))

    qdT = featT(q_data, 0.125)  # [B, 128, 2, Q]

    def featTb(x):  # [B, S, A] -> [B, 128, A//128, S] bf16
        b, sl, a = x.shape
        t = x.transpose(0, 2, 1).reshape(b, a // 128, 128, sl).transpose(0, 2, 1, 3)
        return np.ascontiguousarray(t.astype(BF16))

    mdTb = featTb(m_data)

    # fused[b, h] = (bias[b] + nb[h])^T in chunk layout [128, 8, 3, Q]
    biasT = bias[:, 0].transpose(0, 2, 1).astype(np.float32)  # [B, K, Q]
    nbT = nonbatched_bias.transpose(0, 2, 1).astype(np.float32)  # [H, K, Q]
    inT = np.empty((B, 128, 50, Q), dtype=F8)
    inT[:, :, 0:2, :] = qdT
    for b in range(B):
        f = biasT[b][None] + nbT  # [H, K, Q]
        # hi: 0.5-granular (exact in fp8e4), lo: remainder <= 0.25
        fhi = np.clip(np.round(f * 2.0) / 2.0, -8.0, 8.0)
        flo = f - fhi
        # [H, 3, 128, Q] -> [128, H, 3, Q]
        fc = fhi.reshape(H, 3, 128, Q).transpose(2, 0, 1, 3)
        inT[b, :, 2:26, :] = fc.reshape(128, 24, Q).astype(F8)
        fc = flo.reshape(H, 3, 128, Q).transpose(2, 0, 1, 3)
        inT[b, :, 26:50, :] = fc.reshape(128, 24, Q).astype(F8)

    def wmat(w, s):  # [A, H, hd] -> [128, 2, 256]
        m = (w.reshape(A, H * HD) * s).astype(F8)
        return np.ascontiguousarray(m.reshape(2, 128, 256).transpose(1, 0, 2))

    def wmatb(w):  # [A, H, hd] -> [128, 2, 256] bf16
        m = w.reshape(A, H * HD).astype(BF16)
        return np.ascontiguousarray(m.reshape(2, 128, 256).transpose(1, 0, 2))

    wq = wmat(q_weights, 64.0 * scale)
    wk = wmatb(k_weights)
    wv = wmatb(v_weights)
    wg = wmat(gating_w, 64.0)
    wo = np.ascontiguousarray(
        o_weights.reshape(256, 256).astype(BF16).reshape(2, 128, 256).transpose(1, 0, 2)
    )
    obias_bf = np.ascontiguousarray(o_bias.astype(BF16).reshape(1, OUT))
    gbh = np.ascontiguousarray(
        (0.5 * gating_b.reshape(H * HD).astype(np.float32)).reshape(2, 128).T
    )
    ind = np.zeros((128, 256), dtype=BF16)
    # sums_bf row i holds head 2*(i%4) + i//4 (gather iterates (r, p))
    for i in range(8):
        h = 2 * (i % 4) + i // 4
        ind[i, 32 * h : 32 * (h + 1)] = 1.0
    ind[127, :] = 1.0  # ones row for the o_bias rank-1 matmul
    id2 = np.zeros((128, 2, 128), dtype=F8)
    id2[:, 0, :] = np.eye(128, dtype=np.float32).astype(F8)
    id2[:, 1, :] = id2[:, 0, :]
    return dict(
        inT=inT, inTb=mdTb, wq=wq, wk=wk, wv=wv, wg=wg, wo=wo,
        obias_bf=obias_bf, gbh=gbh, ind=ind, id2=id2,
    )


_NC_CACHE = {}


def kernel(**inputs):
    from concourse.bass_utils import run_bass_kernel_spmd

    full = _prep_inputs(**{k: np.asarray(v) for k, v in inputs.items()})
    if BPC not in _NC_CACHE:
        _NC_CACHE[BPC] = build(BPC)
    nc = _NC_CACHE[BPC]

    shared = {
        k: full[k]
        for k in ("wq", "wk", "wv", "wg", "wo", "obias_bf", "gbh", "ind", "id2")
    }
    in_maps = []
    for c in range(NUM_CORES):
        sl = slice(c * BPC, (c + 1) * BPC)
        in_maps.append(dict(inT=full["inT"][sl], inTb=full["inTb"][sl], **shared))

    trace = bool(int(os.environ.get("BASS_KERNEL_TRACE", "0")))
    if trace:
        try:
            from antenv.axon_hooks import get_axon_ntff_profile_hook  # noqa: F401
        except Exception:
            trace = False
    import time

    t0 = time.time()
    res = run_bass_kernel_spmd(
        nc, in_maps, core_ids=list(range(NUM_CORES)), trace=trace
    )
    kernel.last_run_wall_s = time.time() - t0
    if trace and res.exec_time_ns is not None:
        print(f"HW exec time: {res.exec_time_ns} ns")
        kernel.last_exec_time_ns = res.exec_time_ns
    out = np.concatenate([r["out"] for r in res.results], axis=0)
    return out.astype(np.float32)
